# revision 5
# baseline (speedup 1.0000x reference)
"""Trainium2 Bass kernel for CompanyOperationEvaluation ('rec' branch).

Data-parallel over batch across 8 NeuronCores. Embedding tables and MLP
weights are replicated; features/ent_idx are sharded along B. All matmuls
run in fp32r (full-rate fp32 on the PE at moving-dim >= 256) with the
activations kept transposed ([feature, batch]) so weights serve as lhsT in
their natural [in, out] layout.

The cross-compress recurrence is collapsed algebraically: with
h1 = a1*h0 + b1*e0 + b_c and e1 = g1*h0 + d1*e0 + b_e (per-row scalars from
dot products), the only tensor the MLP head needs is
e2 = A*h0 + B*e0 + C, where A, B, C derive from six per-row dot products
(h0/e0 against w_cf/w_ef/w_fe/w_fc) plus column sums of w_ef/w_fe.
"""

import numpy as np

B, F, D = 16384, 256, 128
H0, H1, OUT = 512, 256, 8
VOCAB = 100000
NCORES = 8
BC = B // NCORES       # 2048 rows per core
NT = BC // 128         # 16 tiles of 128 rows
NG = BC // 512         # 4 groups of 512 rows
GT = 512 // 128        # 4 tiles per group

_CACHE = {}


def _build():
    import concourse.bacc as bacc
    import concourse.bass as bass
    import concourse.tile as tile
    from concourse import mybir
    from concourse.masks import make_identity

    f32 = mybir.dt.float32
    f32r = mybir.dt.float32r
    i32 = mybir.dt.int32
    AF = mybir.ActivationFunctionType
    OP = mybir.AluOpType
    AX = mybir.AxisListType

    nc = bacc.Bacc()

    featT = nc.dram_tensor("featT", (F, BC), f32r, kind="ExternalInput")
    idx = nc.dram_tensor("idx", (BC, 1), i32, kind="ExternalInput")
    head_tab = nc.dram_tensor("head_tab", (VOCAB, D), f32, kind="ExternalInput")
    ent_tab = nc.dram_tensor("ent_tab", (VOCAB, D), f32, kind="ExternalInput")
    Wf = nc.dram_tensor("Wf", (F, D), f32r, kind="ExternalInput")
    Wu = nc.dram_tensor("Wu", (D, D), f32r, kind="ExternalInput")
    W0 = nc.dram_tensor("W0", (2 * D, H0), f32r, kind="ExternalInput")
    W1 = nc.dram_tensor("W1", (H0, H1), f32r, kind="ExternalInput")
    W2 = nc.dram_tensor("W2", (H1, OUT), f32r, kind="ExternalInput")
    # columns: [w_fc, w_fe, w_ef, w_cf, w_ef, w_fe]
    wHE = nc.dram_tensor("wHE", (D, 6), f32r, kind="ExternalInput")
    bfu = nc.dram_tensor("bfu", (D, 2), f32, kind="ExternalInput")
    b0r = nc.dram_tensor("b0r", (D, 4), f32, kind="ExternalInput")
    b1r = nc.dram_tensor("b1r", (D, 2), f32, kind="ExternalInput")
    b2r = nc.dram_tensor("b2r", (OUT, 1), f32, kind="ExternalInput")
    bce = nc.dram_tensor("bce", (D, 2), f32, kind="ExternalInput")
    prob = nc.dram_tensor("prob", (BC, OUT), f32, kind="ExternalOutput")

    with tile.TileContext(nc) as tc:
        with (
            tc.tile_pool(name="pers", bufs=1) as pers,
            tc.tile_pool(name="work", bufs=3) as work,
            tc.tile_pool(name="ps", bufs=3, space="PSUM") as psp,
        ):
            # ---- persistent weights / constants ----
            wf_t = pers.tile([128, F // 128, D], f32r, tag="wf")
            nc.sync.dma_start(out=wf_t[:], in_=Wf.rearrange("(a p) d -> p a d", p=128))
            wu_t = pers.tile([128, D], f32r, tag="wu")
            nc.sync.dma_start(out=wu_t[:], in_=Wu[:])
            w0_t = pers.tile([128, 2, H0], f32r, tag="w0")
            nc.sync.dma_start(out=w0_t[:], in_=W0.rearrange("(a p) h -> p a h", p=128))
            w1_t = pers.tile([128, H0 // 128, H1], f32r, tag="w1")
            nc.sync.dma_start(out=w1_t[:], in_=W1.rearrange("(a p) h -> p a h", p=128))
            w2_t = pers.tile([128, H1 // 128, OUT], f32r, tag="w2")
            nc.sync.dma_start(out=w2_t[:], in_=W2.rearrange("(a p) o -> p a o", p=128))
            whe_t = pers.tile([128, 6], f32r, tag="whe")
            nc.sync.dma_start(out=whe_t[:], in_=wHE[:])
            bfu_t = pers.tile([128, 2], f32, tag="bfu")
            nc.sync.dma_start(out=bfu_t[:], in_=bfu[:])
            b0_t = pers.tile([128, 4], f32, tag="b0")
            nc.sync.dma_start(out=b0_t[:], in_=b0r[:])
            b1_t = pers.tile([128, 2], f32, tag="b1")
            nc.sync.dma_start(out=b1_t[:], in_=b1r[:])
            b2_t = pers.tile([OUT, 1], f32, tag="b2")
            nc.sync.dma_start(out=b2_t[:], in_=b2r[:])
            bce_t = pers.tile([128, 2], f32, tag="bce")
            nc.sync.dma_start(out=bce_t[:], in_=bce[:])

            ident = pers.tile([128, 128], f32, tag="ident")
            make_identity(nc, ident[:])

            xT = pers.tile([128, F // 128, BC], f32r, tag="xT")
            nc.sync.dma_start(out=xT[:], in_=featT.rearrange("(a p) b -> p a b", p=128))

            # ---- column sums of wHE, broadcast to all partitions ----
            ones_c = pers.tile([128, 1], f32, tag="ones_c")
            nc.vector.memset(ones_c[:], 1.0)
            ones_r = pers.tile([1, 128], f32, tag="ones_r")
            nc.vector.memset(ones_r[:], 1.0)
            sums_ps = psp.tile([1, 6], f32, tag="x3ps", bufs=2)
            nc.tensor.matmul(out=sums_ps[:], lhsT=ones_c[:], rhs=whe_t[:].bitcast(f32),
                             start=True, stop=True)
            sums_sb = pers.tile([1, 6], f32, tag="sums")
            nc.scalar.activation(out=sums_sb[:], in_=sums_ps[:], func=AF.Copy)
            sb_ps = psp.tile([128, 6], f32, tag="x3ps", bufs=2)
            nc.tensor.matmul(out=sb_ps[:], lhsT=ones_r[:], rhs=sums_sb[:],
                             start=True, stop=True)
            sb_t = pers.tile([128, 6], f32, tag="sb")
            nc.scalar.activation(out=sb_t[:], in_=sb_ps[:], func=AF.Copy)
            # cef = b_e * sum(w_ef); cfe = b_c * sum(w_fe)
            cef = pers.tile([128, 1], f32, tag="cef")
            nc.vector.tensor_tensor(out=cef[:], in0=sb_t[:, 2:3], in1=bce_t[:, 1:2],
                                    op=OP.mult)
            cfe = pers.tile([128, 1], f32, tag="cfe")
            nc.vector.tensor_tensor(out=cfe[:], in0=sb_t[:, 1:2], in1=bce_t[:, 0:1],
                                    op=OP.mult)

            # ---- gather + transpose + per-row dot products ----
            h0_all = pers.tile([128, BC], f32, tag="h0")
            e0_all = pers.tile([128, BC], f32, tag="e0")
            dots = pers.tile([128, NT * 8], f32, tag="dots")
            for t in range(NT):
                ix_t = work.tile([128, 1], i32, tag="ix")
                nc.sync.dma_start(out=ix_t[:], in_=idx[t * 128:(t + 1) * 128, :])
                bs = slice(t * 128, (t + 1) * 128)
                nc.gpsimd.indirect_dma_start(
                    out=h0_all[:, bs], out_offset=None, in_=head_tab[:],
                    in_offset=bass.IndirectOffsetOnAxis(ap=ix_t[:, :1], axis=0))
                nc.gpsimd.indirect_dma_start(
                    out=e0_all[:, bs], out_offset=None, in_=ent_tab[:],
                    in_offset=bass.IndirectOffsetOnAxis(ap=ix_t[:, :1], axis=0))
                # transpose h0/e0 tile -> [d, b] (fp32r for the dot matmuls)
                hT_ps = psp.tile([128, 128], f32, tag="trps")
                nc.tensor.transpose(out=hT_ps[:], in_=h0_all[:, bs], identity=ident[:])
                hT = work.tile([128, 128], f32r, tag="hT")
                nc.scalar.activation(out=hT[:], in_=hT_ps[:], func=AF.Copy)
                eT_ps = psp.tile([128, 128], f32, tag="trps")
                nc.tensor.transpose(out=eT_ps[:], in_=e0_all[:, bs], identity=ident[:])
                eT = work.tile([128, 128], f32r, tag="eT")
                nc.scalar.activation(out=eT[:], in_=eT_ps[:], func=AF.Copy)
                # dots (fp32r needs even moving dim -> N=4 each):
                # cols 0..3 = h0 . (w_fc, w_fe, w_ef, w_cf)
                # cols 4..7 = e0 . (w_ef, w_cf, w_ef, w_fe)
                d_ps = psp.tile([128, 8], f32, tag="trps")
                nc.tensor.matmul(out=d_ps[:, 0:4], lhsT=hT[:], rhs=whe_t[:, 0:4],
                                 start=True, stop=True)
                nc.tensor.matmul(out=d_ps[:, 4:8], lhsT=eT[:], rhs=whe_t[:, 2:6],
                                 start=True, stop=True)
                nc.vector.tensor_copy(out=dots[:, t * 8:(t + 1) * 8], in_=d_ps[:, 0:8])

            # ---- coefficients A, B, C  (views strided across tiles) ----
            dv = dots.rearrange("p (t c) -> p t c", c=8)
            c0, c1, c2 = dv[:, :, 0], dv[:, :, 1], dv[:, :, 2]  # H_fc, H_fe, H_ef
            c3, c4, c5 = dv[:, :, 5], dv[:, :, 4], dv[:, :, 7]  # E_cf, E_ef, E_fe

            def tt(out, a, b, op):
                nc.vector.tensor_tensor(out=out, in0=a, in1=b, op=op)

            t1 = pers.tile([128, NT], f32, tag="t1")
            t2 = pers.tile([128, NT], f32, tag="t2")
            a2 = pers.tile([128, NT], f32, tag="a2")
            d2 = pers.tile([128, NT], f32, tag="d2")
            A = pers.tile([128, NT], f32, tag="A")
            Bc = pers.tile([128, NT], f32, tag="B")
            Cc = pers.tile([128, NT], f32, tag="C")
            # a2 = c4*c2 + c1*c4 + cef
            tt(t1[:], c4, c2, OP.mult)
            tt(t2[:], c1, c4, OP.mult)
            tt(a2[:], t1[:], t2[:], OP.add)
            nc.vector.tensor_scalar(out=a2[:], in0=a2[:], scalar1=cef[:, 0:1],
                                    scalar2=None, op0=OP.add)
            # d2 = c3*c1 + c0*c5 + cfe
            tt(t1[:], c3, c1, OP.mult)
            tt(t2[:], c0, c5, OP.mult)
            tt(d2[:], t1[:], t2[:], OP.add)
            nc.vector.tensor_scalar(out=d2[:], in0=d2[:], scalar1=cfe[:, 0:1],
                                    scalar2=None, op0=OP.add)
            # A = a2*c3 + d2*c4 ; B = a2*c0 + d2*c1
            tt(t1[:], a2[:], c3, OP.mult)
            tt(t2[:], d2[:], c4, OP.mult)
            tt(A[:], t1[:], t2[:], OP.add)
            tt(t1[:], a2[:], c0, OP.mult)
            tt(t2[:], d2[:], c1, OP.mult)
            tt(Bc[:], t1[:], t2[:], OP.add)
            # C = a2*b_c + (d2*b_e + b_e)
            nc.vector.tensor_scalar(out=t1[:], in0=a2[:], scalar1=bce_t[:, 0:1],
                                    scalar2=None, op0=OP.mult)
            nc.vector.tensor_scalar(out=t2[:], in0=d2[:], scalar1=bce_t[:, 1:2],
                                    scalar2=bce_t[:, 1:2], op0=OP.mult, op1=OP.add)
            tt(Cc[:], t1[:], t2[:], OP.add)

            # ---- e2 = A*h0 + B*e0 + C, transposed into [d, b] ----
            e2T = pers.tile([128, BC], f32r, tag="e2T")
            for t in range(NT):
                bs = slice(t * 128, (t + 1) * 128)
                m1 = work.tile([128, 128], f32, tag="m1")
                nc.vector.tensor_scalar(out=m1[:], in0=h0_all[:, bs],
                                        scalar1=A[:, t:t + 1], scalar2=Cc[:, t:t + 1],
                                        op0=OP.mult, op1=OP.add)
                m2 = work.tile([128, 128], f32, tag="m2")
                nc.vector.tensor_scalar(out=m2[:], in0=e0_all[:, bs],
                                        scalar1=Bc[:, t:t + 1], scalar2=None,
                                        op0=OP.mult)
                e2n = work.tile([128, 128], f32, tag="e2n")
                tt(e2n[:], m1[:], m2[:], OP.add)
                e2_ps = psp.tile([128, 128], f32, tag="trps")
                nc.tensor.transpose(out=e2_ps[:], in_=e2n[:], identity=ident[:])
                nc.scalar.activation(out=e2T[:, bs], in_=e2_ps[:], func=AF.Copy)

            # ---- main MLP per 512-column group ----
            for g in range(NG):
                gs = slice(g * 512, (g + 1) * 512)
                cf_ps = psp.tile([128, 512], f32, tag="mmps")
                nc.tensor.matmul(out=cf_ps[:], lhsT=wf_t[:, 0, :], rhs=xT[:, 0, gs],
                                 start=True, stop=False)
                nc.tensor.matmul(out=cf_ps[:], lhsT=wf_t[:, 1, :], rhs=xT[:, 1, gs],
                                 start=False, stop=True)
                cf = work.tile([128, 512], f32r, tag="cf")
                nc.scalar.activation(out=cf[:], in_=cf_ps[:], func=AF.Relu,
                                     bias=bfu_t[:, 0:1])
                for _ in range(2):
                    cu_ps = psp.tile([128, 512], f32, tag="mmps")
                    nc.tensor.matmul(out=cu_ps[:], lhsT=wu_t[:], rhs=cf[:],
                                     start=True, stop=True)
                    cf = work.tile([128, 512], f32r, tag="cf")
                    nc.scalar.activation(out=cf[:], in_=cu_ps[:], func=AF.Relu,
                                         bias=bfu_t[:, 1:2])
                x1 = work.tile([128, 4 * 512], f32r, tag="x1")
                for m in range(4):
                    x1_ps = psp.tile([128, 512], f32, tag="mmps")
                    ms = slice(m * 128, (m + 1) * 128)
                    nc.tensor.matmul(out=x1_ps[:], lhsT=w0_t[:, 0, ms], rhs=cf[:],
                                     start=True, stop=False)
                    nc.tensor.matmul(out=x1_ps[:], lhsT=w0_t[:, 1, ms], rhs=e2T[:, gs],
                                     start=False, stop=True)
                    nc.scalar.activation(out=x1[:, m * 512:(m + 1) * 512], in_=x1_ps[:],
                                         func=AF.Relu, bias=b0_t[:, m:m + 1])
                x2 = work.tile([128, 2 * 512], f32r, tag="x2")
                for m in range(2):
                    x2_ps = psp.tile([128, 512], f32, tag="mmps")
                    ms = slice(m * 128, (m + 1) * 128)
                    for k in range(4):
                        nc.tensor.matmul(out=x2_ps[:], lhsT=w1_t[:, k, ms],
                                         rhs=x1[:, k * 512:(k + 1) * 512],
                                         start=(k == 0), stop=(k == 3))
                    nc.scalar.activation(out=x2[:, m * 512:(m + 1) * 512], in_=x2_ps[:],
                                         func=AF.Relu, bias=b1_t[:, m:m + 1])
                x3_ps = psp.tile([OUT, 512], f32, tag="x3ps", bufs=2)
                for k in range(2):
                    nc.tensor.matmul(out=x3_ps[:], lhsT=w2_t[:, k, :],
                                     rhs=x2[:, k * 512:(k + 1) * 512],
                                     start=(k == 0), stop=(k == 1))
                x3 = work.tile([OUT, 512], f32, tag="x3")
                nc.scalar.activation(out=x3[:], in_=x3_ps[:], func=AF.Relu,
                                     bias=b2_t[:, 0:1])
                # softmax over OUT=8: transpose back to [b, 8] in 128-row blocks
                ex = work.tile([128, GT * OUT], f32, tag="ex")
                for j in range(GT):
                    sm_ps = psp.tile([128, OUT], f32, tag="trps")
                    nc.tensor.transpose(out=sm_ps[:], in_=x3[:, j * 128:(j + 1) * 128],
                                        identity=ident[:OUT, :OUT])
                    nc.scalar.activation(out=ex[:, j * OUT:(j + 1) * OUT], in_=sm_ps[:],
                                         func=AF.Exp)
                sm_sum = work.tile([128, GT], f32, tag="sm_sum")
                nc.vector.reduce_sum(out=sm_sum[:],
                                     in_=ex.rearrange("p (j o) -> p j o", o=OUT),
                                     axis=AX.X)
                sm_rec = work.tile([128, GT], f32, tag="sm_rec")
                nc.vector.reciprocal(out=sm_rec[:], in_=sm_sum[:])
                pr = work.tile([128, GT * OUT], f32, tag="pr")
                nc.vector.tensor_tensor(
                    out=pr.rearrange("p (j o) -> p j o", o=OUT),
                    in0=ex.rearrange("p (j o) -> p j o", o=OUT),
                    in1=sm_rec.rearrange("p (j o) -> p j o", o=1).to_broadcast(
                        [128, GT, OUT]),
                    op=OP.mult)
                nc.sync.dma_start(
                    out=prob[g * 512:(g + 1) * 512, :].rearrange(
                        "(j p) o -> p j o", p=128),
                    in_=pr.rearrange("p (j o) -> p j o", o=OUT))

    nc.finalize()
    return nc


def _get_nc():
    if "nc" not in _CACHE:
        _CACHE["nc"] = _build()
    return _CACHE["nc"]


def kernel(features, ent_idx, target, Wf, bf, Wu, bu, w_cf, w_fc, w_ef, w_fe,
           b_c, b_e, head_tab, ent_tab, W0, b0, W1, b1, W2, b2):
    from concourse.bass_utils import run_bass_kernel_spmd

    features = np.asarray(features, dtype=np.float32)
    ent_idx = np.asarray(ent_idx)
    target = np.asarray(target)
    f32 = np.float32
    head_tab = np.asarray(head_tab, dtype=f32)
    ent_tab = np.asarray(ent_tab, dtype=f32)
    w_cf, w_fc = np.asarray(w_cf, f32), np.asarray(w_fc, f32)
    w_ef, w_fe = np.asarray(w_ef, f32), np.asarray(w_fe, f32)

    featT = np.ascontiguousarray(features.T)                    # [F, B]
    idx_all = ent_idx.astype(np.int32).reshape(B, 1)
    wHE = np.ascontiguousarray(
        np.stack([w_fc, w_fe, w_ef, w_cf, w_ef, w_fe], axis=1))  # [D, 6]
    bfu = np.ascontiguousarray(
        np.stack([np.asarray(bf, f32), np.asarray(bu, f32)], axis=1))
    b0r = np.ascontiguousarray(np.asarray(b0, f32).reshape(4, D).T)
    b1r = np.ascontiguousarray(np.asarray(b1, f32).reshape(2, D).T)
    b2r = np.ascontiguousarray(np.asarray(b2, f32).reshape(OUT, 1))
    bce = np.ascontiguousarray(np.broadcast_to(
        np.array([np.asarray(b_c, f32).reshape(()),
                  np.asarray(b_e, f32).reshape(())], dtype=f32), (D, 2)))
    shared = dict(
        head_tab=head_tab, ent_tab=ent_tab,
        Wf=np.asarray(Wf, f32), Wu=np.asarray(Wu, f32),
        W0=np.asarray(W0, f32), W1=np.asarray(W1, f32), W2=np.asarray(W2, f32),
        wHE=wHE, bfu=bfu, b0r=b0r, b1r=b1r, b2r=b2r, bce=bce,
    )
    in_maps = []
    for c in range(NCORES):
        cs = slice(c * BC, (c + 1) * BC)
        in_maps.append(dict(
            featT=np.ascontiguousarray(featT[:, cs]),
            idx=idx_all[cs], **shared))

    nc = _get_nc()
    res = run_bass_kernel_spmd(nc, in_maps, core_ids=list(range(NCORES)))
    prob = np.concatenate([r["prob"] for r in res.results], axis=0)
    return prob, target


# revision 8
# speedup vs baseline: 1.4348x; 1.4348x over previous
"""Trainium2 Bass kernel for CompanyOperationEvaluation ('rec' branch).

Data-parallel over batch across 8 NeuronCores. Embedding tables and MLP
weights are replicated; features/ent_idx are sharded along B. All matmuls
run in fp32r (full-rate fp32 on the PE at moving-dim >= 256) with the
activations kept transposed ([feature, batch]) so weights serve as lhsT in
their natural [in, out] layout.

The cross-compress recurrence is collapsed algebraically: with
h1 = a1*h0 + b1*e0 + b_c and e1 = g1*h0 + d1*e0 + b_e (per-row scalars from
dot products), the only tensor the MLP head needs is
e2 = A*h0 + B*e0 + C, where A, B, C derive from six per-row dot products
(h0/e0 against w_cf/w_ef/w_fe/w_fc) plus column sums of w_ef/w_fe.

Scheduling notes: the PE executes its stream in order, so the main MLP is
emitted layer-major across all four 512-column batch groups to keep
independent matmuls available while activations (ACT/DVE) chase the
dependent relus. Embedding rows are fetched with a single 2048-descriptor
indirect DMA per table.
"""

import numpy as np

B, F, D = 16384, 256, 128
H0, H1, OUT = 512, 256, 8
VOCAB = 100000
NCORES = 8
BC = B // NCORES       # 2048 rows per core
NT = BC // 128         # 16 tiles of 128 rows
NG = BC // 512         # 4 groups of 512 rows
GT = 512 // 128        # 4 tiles per group

_CACHE = {}


def _build():
    import concourse.bacc as bacc
    import concourse.bass as bass
    import concourse.tile as tile
    from concourse import mybir
    from concourse.masks import make_identity

    f32 = mybir.dt.float32
    f32r = mybir.dt.float32r
    i32 = mybir.dt.int32
    AF = mybir.ActivationFunctionType
    OP = mybir.AluOpType
    AX = mybir.AxisListType

    nc = bacc.Bacc()

    featT = nc.dram_tensor("featT", (F, BC), f32r, kind="ExternalInput")
    # idx2[p, t] = ent_idx[t*128 + p]
    idx2 = nc.dram_tensor("idx2", (128, NT), i32, kind="ExternalInput")
    tabs = nc.dram_tensor("tabs", (VOCAB, 2 * D), f32, kind="ExternalInput")
    Wf = nc.dram_tensor("Wf", (F, D), f32r, kind="ExternalInput")
    Wu = nc.dram_tensor("Wu", (D, D), f32r, kind="ExternalInput")
    W0 = nc.dram_tensor("W0", (2 * D, H0), f32r, kind="ExternalInput")
    W1 = nc.dram_tensor("W1", (H0, H1), f32r, kind="ExternalInput")
    W2 = nc.dram_tensor("W2", (H1, OUT), f32r, kind="ExternalInput")
    # columns: [w_fc, w_fe, w_ef, w_cf, w_ef, w_fe]
    wHE = nc.dram_tensor("wHE", (D, 6), f32r, kind="ExternalInput")
    bfu = nc.dram_tensor("bfu", (D, 2), f32, kind="ExternalInput")
    b0r = nc.dram_tensor("b0r", (D, 4), f32, kind="ExternalInput")
    b1r = nc.dram_tensor("b1r", (D, 2), f32, kind="ExternalInput")
    b2r = nc.dram_tensor("b2r", (OUT, 1), f32, kind="ExternalInput")
    bce = nc.dram_tensor("bce", (D, 2), f32, kind="ExternalInput")
    prob = nc.dram_tensor("prob", (BC, OUT), f32, kind="ExternalOutput")

    with tile.TileContext(nc) as tc:
        with (
            tc.tile_pool(name="pers", bufs=1) as pers,
            tc.tile_pool(name="work", bufs=3) as work,
            tc.tile_pool(name="ps", bufs=4, space="PSUM") as psp,
        ):
            # ---- persistent weights / constants ----
            wf_t = pers.tile([128, F // 128, D], f32r, tag="wf")
            nc.sync.dma_start(out=wf_t[:], in_=Wf.rearrange("(a p) d -> p a d", p=128))
            wu_t = pers.tile([128, D], f32r, tag="wu")
            nc.sync.dma_start(out=wu_t[:], in_=Wu[:])
            w0_t = pers.tile([128, 2, H0], f32r, tag="w0")
            nc.sync.dma_start(out=w0_t[:], in_=W0.rearrange("(a p) h -> p a h", p=128))
            w1_t = pers.tile([128, H0 // 128, H1], f32r, tag="w1")
            nc.sync.dma_start(out=w1_t[:], in_=W1.rearrange("(a p) h -> p a h", p=128))
            w2_t = pers.tile([128, H1 // 128, OUT], f32r, tag="w2")
            nc.sync.dma_start(out=w2_t[:], in_=W2.rearrange("(a p) o -> p a o", p=128))
            whe_t = pers.tile([128, 6], f32r, tag="whe")
            nc.sync.dma_start(out=whe_t[:], in_=wHE[:])
            bfu_t = pers.tile([128, 2], f32, tag="bfu")
            nc.sync.dma_start(out=bfu_t[:], in_=bfu[:])
            b0_t = pers.tile([128, 4], f32, tag="b0")
            nc.sync.dma_start(out=b0_t[:], in_=b0r[:])
            b1_t = pers.tile([128, 2], f32, tag="b1")
            nc.sync.dma_start(out=b1_t[:], in_=b1r[:])
            b2_t = pers.tile([OUT, 1], f32, tag="b2")
            nc.sync.dma_start(out=b2_t[:], in_=b2r[:])
            bce_t = pers.tile([128, 2], f32, tag="bce")
            nc.sync.dma_start(out=bce_t[:], in_=bce[:])

            ident = pers.tile([128, 128], f32, tag="ident")
            make_identity(nc, ident[:])

            # ---- per-tile gathers; head/ent interleaved -> 1KB per descriptor ----
            ix_t = pers.tile([128, NT], i32, tag="ix")
            nc.sync.dma_start(out=ix_t[:], in_=idx2[:])
            he_all = pers.tile([128, NT * 2 * D], f32, tag="he")
            for t in range(NT):
                nc.gpsimd.indirect_dma_start(
                    out=he_all[:, t * 2 * D:(t + 1) * 2 * D],
                    out_offset=None, in_=tabs[:],
                    in_offset=bass.IndirectOffsetOnAxis(ap=ix_t[:, t:t + 1], axis=0))

            xT = pers.tile([128, F // 128, BC], f32r, tag="xT")
            nc.sync.dma_start(out=xT[:], in_=featT.rearrange("(a p) b -> p a b", p=128))

            # ---- column sums of wHE, broadcast to all partitions ----
            ones_c = pers.tile([128, 1], f32, tag="ones_c")
            nc.vector.memset(ones_c[:], 1.0)
            ones_r = pers.tile([1, 128], f32, tag="ones_r")
            nc.vector.memset(ones_r[:], 1.0)
            sums_ps = psp.tile([1, 6], f32, tag="smps", bufs=2)
            nc.tensor.matmul(out=sums_ps[:], lhsT=ones_c[:], rhs=whe_t[:].bitcast(f32),
                             start=True, stop=True)
            sums_sb = pers.tile([1, 6], f32, tag="sums")
            nc.scalar.activation(out=sums_sb[:], in_=sums_ps[:], func=AF.Copy)
            sb_ps = psp.tile([128, 6], f32, tag="smps", bufs=2)
            nc.tensor.matmul(out=sb_ps[:], lhsT=ones_r[:], rhs=sums_sb[:],
                             start=True, stop=True)
            sb_t = pers.tile([128, 6], f32, tag="sb")
            nc.scalar.activation(out=sb_t[:], in_=sb_ps[:], func=AF.Copy)
            # cef = b_e * sum(w_ef); cfe = b_c * sum(w_fe)
            cef = pers.tile([128, 1], f32, tag="cef")
            nc.vector.tensor_tensor(out=cef[:], in0=sb_t[:, 2:3], in1=bce_t[:, 1:2],
                                    op=OP.mult)
            cfe = pers.tile([128, 1], f32, tag="cfe")
            nc.vector.tensor_tensor(out=cfe[:], in0=sb_t[:, 1:2], in1=bce_t[:, 0:1],
                                    op=OP.mult)

            # ---- transpose h0/e0 into [d, b]; batched PSUM -> one copy per 512 ----
            def h0s(t):
                return he_all[:, t * 2 * D:t * 2 * D + D]

            def e0s(t):
                return he_all[:, t * 2 * D + D:(t + 1) * 2 * D]

            hT_all = pers.tile([128, BC], f32r, tag="hT")
            eT_all = pers.tile([128, BC], f32r, tag="eT")
            for g in range(NG):
                gs = slice(g * 512, (g + 1) * 512)
                for sel, dst in ((h0s, hT_all), (e0s, eT_all)):
                    tp = psp.tile([128, 512], f32, tag="trps", bufs=2)
                    for j in range(GT):
                        t = g * GT + j
                        nc.tensor.transpose(
                            out=tp[:, j * 128:(j + 1) * 128],
                            in_=sel(t), identity=ident[:])
                    nc.scalar.activation(out=dst[:, gs], in_=tp[:], func=AF.Copy)

            # ---- per-row dot products (fp32r needs even moving dim -> N=4) ----
            # cols 0..3 = h0 . (w_fc, w_fe, w_ef, w_cf)
            # cols 4..7 = e0 . (w_ef, w_cf, w_ef, w_fe)
            dots = pers.tile([128, NT * 8], f32, tag="dots")
            for t in range(NT):
                bs = slice(t * 128, (t + 1) * 128)
                d_ps = psp.tile([128, 8], f32, tag="smps", bufs=2)
                nc.tensor.matmul(out=d_ps[:, 0:4], lhsT=hT_all[:, bs],
                                 rhs=whe_t[:, 0:4], start=True, stop=True)
                nc.tensor.matmul(out=d_ps[:, 4:8], lhsT=eT_all[:, bs],
                                 rhs=whe_t[:, 2:6], start=True, stop=True)
                nc.vector.tensor_copy(out=dots[:, t * 8:(t + 1) * 8], in_=d_ps[:, 0:8])

            # ---- coefficients A, B, C  (views strided across tiles) ----
            dv = dots.rearrange("p (t c) -> p t c", c=8)
            c0, c1, c2 = dv[:, :, 0], dv[:, :, 1], dv[:, :, 2]  # H_fc, H_fe, H_ef
            c3, c4, c5 = dv[:, :, 5], dv[:, :, 4], dv[:, :, 7]  # E_cf, E_ef, E_fe

            def tt(out, a, b, op):
                nc.vector.tensor_tensor(out=out, in0=a, in1=b, op=op)

            t1 = pers.tile([128, NT], f32, tag="t1")
            t2 = pers.tile([128, NT], f32, tag="t2")
            a2 = pers.tile([128, NT], f32, tag="a2")
            d2 = pers.tile([128, NT], f32, tag="d2")
            A = pers.tile([128, NT], f32, tag="A")
            Bc = pers.tile([128, NT], f32, tag="B")
            Cc = pers.tile([128, NT], f32, tag="C")
            # a2 = c4*c2 + c1*c4 + cef
            tt(t1[:], c4, c2, OP.mult)
            tt(t2[:], c1, c4, OP.mult)
            tt(a2[:], t1[:], t2[:], OP.add)
            nc.vector.tensor_scalar(out=a2[:], in0=a2[:], scalar1=cef[:, 0:1],
                                    scalar2=None, op0=OP.add)
            # d2 = c3*c1 + c0*c5 + cfe
            tt(t1[:], c3, c1, OP.mult)
            tt(t2[:], c0, c5, OP.mult)
            tt(d2[:], t1[:], t2[:], OP.add)
            nc.vector.tensor_scalar(out=d2[:], in0=d2[:], scalar1=cfe[:, 0:1],
                                    scalar2=None, op0=OP.add)
            # A = a2*c3 + d2*c4 ; B = a2*c0 + d2*c1
            tt(t1[:], a2[:], c3, OP.mult)
            tt(t2[:], d2[:], c4, OP.mult)
            tt(A[:], t1[:], t2[:], OP.add)
            tt(t1[:], a2[:], c0, OP.mult)
            tt(t2[:], d2[:], c1, OP.mult)
            tt(Bc[:], t1[:], t2[:], OP.add)
            # C = a2*b_c + (d2*b_e + b_e)
            nc.vector.tensor_scalar(out=t1[:], in0=a2[:], scalar1=bce_t[:, 0:1],
                                    scalar2=None, op0=OP.mult)
            nc.vector.tensor_scalar(out=t2[:], in0=d2[:], scalar1=bce_t[:, 1:2],
                                    scalar2=bce_t[:, 1:2], op0=OP.mult, op1=OP.add)
            tt(Cc[:], t1[:], t2[:], OP.add)

            # ---- e2 = A*h0 + B*e0 + C, transposed into [d, b] ----
            e2T = pers.tile([128, BC], f32r, tag="e2T")
            for g in range(NG):
                tp = psp.tile([128, 512], f32, tag="trps", bufs=2)
                for j in range(GT):
                    t = g * GT + j
                    m1 = work.tile([128, 128], f32, tag="m1")
                    nc.vector.tensor_scalar(out=m1[:], in0=h0s(t),
                                            scalar1=A[:, t:t + 1],
                                            scalar2=Cc[:, t:t + 1],
                                            op0=OP.mult, op1=OP.add)
                    e2n = work.tile([128, 128], f32, tag="e2n")
                    nc.vector.tensor_scalar(out=e2n[:], in0=e0s(t),
                                            scalar1=Bc[:, t:t + 1], scalar2=None,
                                            op0=OP.mult)
                    tt(e2n[:], m1[:], e2n[:], OP.add)
                    nc.tensor.transpose(out=tp[:, j * 128:(j + 1) * 128],
                                        in_=e2n[:], identity=ident[:])
                nc.scalar.activation(out=e2T[:, g * 512:(g + 1) * 512], in_=tp[:],
                                     func=AF.Copy)

            # ---- main MLP, layer-major across groups to keep the PE fed ----
            GS = [slice(g * 512, (g + 1) * 512) for g in range(NG)]

            def relu(dst, src, bias_ap, on_vector):
                if on_vector:
                    nc.vector.tensor_scalar(out=dst, in0=src, scalar1=bias_ap,
                                            scalar2=0.0, op0=OP.add, op1=OP.max)
                else:
                    nc.scalar.activation(out=dst, in_=src, func=AF.Relu, bias=bias_ap)

            cf0 = pers.tile([128, BC], f32r, tag="cf0")
            cf1 = pers.tile([128, BC], f32r, tag="cf1")
            cf2 = pers.tile([128, BC], f32r, tag="cf2")
            x1a = pers.tile([128, NG, 4 * 512], f32r, tag="x1a")
            x2a = pers.tile([128, NG, 2 * 512], f32r, tag="x2a")
            x3a = pers.tile([OUT, NG, 512], f32, tag="x3a")
            for g in range(NG):
                cf_ps = psp.tile([128, 512], f32, tag="mmps", bufs=4)
                nc.tensor.matmul(out=cf_ps[:], lhsT=wf_t[:, 0, :], rhs=xT[:, 0, GS[g]],
                                 start=True, stop=False)
                nc.tensor.matmul(out=cf_ps[:], lhsT=wf_t[:, 1, :], rhs=xT[:, 1, GS[g]],
                                 start=False, stop=True)
                relu(cf0[:, GS[g]], cf_ps[:], bfu_t[:, 0:1], on_vector=(g % 2 == 1))
            for src, dst in ((cf0, cf1), (cf1, cf2)):
                for g in range(NG):
                    cu_ps = psp.tile([128, 512], f32, tag="mmps", bufs=4)
                    nc.tensor.matmul(out=cu_ps[:], lhsT=wu_t[:], rhs=src[:, GS[g]],
                                     start=True, stop=True)
                    relu(dst[:, GS[g]], cu_ps[:], bfu_t[:, 1:2],
                         on_vector=(g % 2 == 1))
            for g in range(NG):
                for m in range(4):
                    x1_ps = psp.tile([128, 512], f32, tag="mmps", bufs=4)
                    ms = slice(m * 128, (m + 1) * 128)
                    nc.tensor.matmul(out=x1_ps[:], lhsT=w0_t[:, 0, ms],
                                     rhs=cf2[:, GS[g]], start=True, stop=False)
                    nc.tensor.matmul(out=x1_ps[:], lhsT=w0_t[:, 1, ms],
                                     rhs=e2T[:, GS[g]], start=False, stop=True)
                    relu(x1a[:, g, m * 512:(m + 1) * 512], x1_ps[:], b0_t[:, m:m + 1],
                         on_vector=(m % 2 == 1))
            for g in range(NG):
                for m in range(2):
                    x2_ps = psp.tile([128, 512], f32, tag="mmps", bufs=4)
                    ms = slice(m * 128, (m + 1) * 128)
                    for k in range(4):
                        nc.tensor.matmul(out=x2_ps[:], lhsT=w1_t[:, k, ms],
                                         rhs=x1a[:, g, k * 512:(k + 1) * 512],
                                         start=(k == 0), stop=(k == 3))
                    relu(x2a[:, g, m * 512:(m + 1) * 512], x2_ps[:], b1_t[:, m:m + 1],
                         on_vector=(m % 2 == 1))
            for g in range(NG):
                x3_ps = psp.tile([OUT, 512], f32, tag="smps", bufs=2)
                for k in range(2):
                    nc.tensor.matmul(out=x3_ps[:], lhsT=w2_t[:, k, :],
                                     rhs=x2a[:, g, k * 512:(k + 1) * 512],
                                     start=(k == 0), stop=(k == 1))
                nc.scalar.activation(out=x3a[:, g, :], in_=x3_ps[:], func=AF.Relu,
                                     bias=b2_t[:, 0:1])
            # softmax over OUT=8: transpose back to [b, 8] in 128-row blocks
            for g in range(NG):
                ex = work.tile([128, GT * OUT], f32, tag="ex")
                for j in range(GT):
                    sm_ps = psp.tile([128, OUT], f32, tag="smps", bufs=2)
                    nc.tensor.transpose(out=sm_ps[:],
                                        in_=x3a[:, g, j * 128:(j + 1) * 128],
                                        identity=ident[:OUT, :OUT])
                    nc.scalar.activation(out=ex[:, j * OUT:(j + 1) * OUT],
                                         in_=sm_ps[:], func=AF.Exp)
                sm_sum = work.tile([128, GT], f32, tag="sm_sum")
                nc.vector.reduce_sum(out=sm_sum[:],
                                     in_=ex.rearrange("p (j o) -> p j o", o=OUT),
                                     axis=AX.X)
                sm_rec = work.tile([128, GT], f32, tag="sm_rec")
                nc.vector.reciprocal(out=sm_rec[:], in_=sm_sum[:])
                pr = work.tile([128, GT * OUT], f32, tag="pr")
                nc.vector.tensor_tensor(
                    out=pr.rearrange("p (j o) -> p j o", o=OUT),
                    in0=ex.rearrange("p (j o) -> p j o", o=OUT),
                    in1=sm_rec.rearrange("p (j o) -> p j o", o=1).to_broadcast(
                        [128, GT, OUT]),
                    op=OP.mult)
                nc.sync.dma_start(
                    out=prob[g * 512:(g + 1) * 512, :].rearrange(
                        "(j p) o -> p j o", p=128),
                    in_=pr.rearrange("p (j o) -> p j o", o=OUT))

    nc.finalize()
    return nc


def _get_nc():
    if "nc" not in _CACHE:
        _CACHE["nc"] = _build()
    return _CACHE["nc"]


def kernel(features, ent_idx, target, Wf, bf, Wu, bu, w_cf, w_fc, w_ef, w_fe,
           b_c, b_e, head_tab, ent_tab, W0, b0, W1, b1, W2, b2):
    from concourse.bass_utils import run_bass_kernel_spmd

    features = np.asarray(features, dtype=np.float32)
    ent_idx = np.asarray(ent_idx)
    target = np.asarray(target)
    f32 = np.float32
    head_tab = np.asarray(head_tab, dtype=f32)
    ent_tab = np.asarray(ent_tab, dtype=f32)
    w_cf, w_fc = np.asarray(w_cf, f32), np.asarray(w_fc, f32)
    w_ef, w_fe = np.asarray(w_ef, f32), np.asarray(w_fe, f32)

    featT = np.ascontiguousarray(features.T)                    # [F, B]
    idx_all = ent_idx.astype(np.int32)
    wHE = np.ascontiguousarray(
        np.stack([w_fc, w_fe, w_ef, w_cf, w_ef, w_fe], axis=1))  # [D, 6]
    bfu = np.ascontiguousarray(
        np.stack([np.asarray(bf, f32), np.asarray(bu, f32)], axis=1))
    b0r = np.ascontiguousarray(np.asarray(b0, f32).reshape(4, D).T)
    b1r = np.ascontiguousarray(np.asarray(b1, f32).reshape(2, D).T)
    b2r = np.ascontiguousarray(np.asarray(b2, f32).reshape(OUT, 1))
    bce = np.ascontiguousarray(np.broadcast_to(
        np.array([np.asarray(b_c, f32).reshape(()),
                  np.asarray(b_e, f32).reshape(())], dtype=f32), (D, 2)))
    tabs = np.ascontiguousarray(np.concatenate([head_tab, ent_tab], axis=1))
    shared = dict(
        tabs=tabs,
        Wf=np.asarray(Wf, f32), Wu=np.asarray(Wu, f32),
        W0=np.asarray(W0, f32), W1=np.asarray(W1, f32), W2=np.asarray(W2, f32),
        wHE=wHE, bfu=bfu, b0r=b0r, b1r=b1r, b2r=b2r, bce=bce,
    )
    in_maps = []
    for c in range(NCORES):
        cs = slice(c * BC, (c + 1) * BC)
        in_maps.append(dict(
            featT=np.ascontiguousarray(featT[:, cs]),
            idx2=np.ascontiguousarray(idx_all[cs].reshape(NT, 128).T),
            **shared))

    nc = _get_nc()
    res = run_bass_kernel_spmd(nc, in_maps, core_ids=list(range(NCORES)))
    prob = np.concatenate([r["prob"] for r in res.results], axis=0)
    return prob, target


# revision 11
# speedup vs baseline: 1.5677x; 1.0927x over previous
"""Trainium2 Bass kernel for CompanyOperationEvaluation ('rec' branch).

Data-parallel over batch across 8 NeuronCores. Embedding tables and MLP
weights are replicated; features/ent_idx are sharded along B. All matmuls
run in fp32r (full-rate fp32 on the PE at moving-dim >= 256) with the
activations kept transposed ([feature, batch]) so weights serve as lhsT in
their natural [in, out] layout.

The cross-compress recurrence is collapsed algebraically: with
h1 = a1*h0 + b1*e0 + b_c and e1 = g1*h0 + d1*e0 + b_e (per-row scalars from
dot products), the only tensor the MLP head needs is
e2 = A*h0 + B*e0 + C, where A, B, C derive from six per-row dot products
(h0/e0 against w_cf/w_ef/w_fe/w_fc) plus column sums of w_ef/w_fe.

Scheduling notes: the PE executes its stream in order, so the main MLP is
emitted layer-major across all four 512-column batch groups to keep
independent matmuls available while activations (ACT/DVE) chase the
dependent relus. Embedding rows are fetched with a single 2048-descriptor
indirect DMA per table.
"""

import numpy as np

B, F, D = 16384, 256, 128
H0, H1, OUT = 512, 256, 8
VOCAB = 100000
NCORES = 8
BC = B // NCORES       # 2048 rows per core
NT = BC // 128         # 16 tiles of 128 rows
NG = BC // 512         # 4 groups of 512 rows
GT = 512 // 128        # 4 tiles per group

_CACHE = {}


def _build():
    import concourse.bacc as bacc
    import concourse.bass as bass
    import concourse.tile as tile
    from concourse import mybir
    from concourse.masks import make_identity

    f32 = mybir.dt.float32
    bf16 = mybir.dt.bfloat16
    i32 = mybir.dt.int32
    AF = mybir.ActivationFunctionType
    OP = mybir.AluOpType
    AX = mybir.AxisListType

    nc = bacc.Bacc()

    featT = nc.dram_tensor("featT", (F, BC), bf16, kind="ExternalInput")
    # idx2[p, t] = ent_idx[t*128 + p]
    idx2 = nc.dram_tensor("idx2", (128, NT), i32, kind="ExternalInput")
    tabs = nc.dram_tensor("tabs", (VOCAB, 2 * D), f32, kind="ExternalInput")
    Wf = nc.dram_tensor("Wf", (F, D), bf16, kind="ExternalInput")
    Wu = nc.dram_tensor("Wu", (D, D), bf16, kind="ExternalInput")
    W0 = nc.dram_tensor("W0", (2 * D, H0), bf16, kind="ExternalInput")
    W1 = nc.dram_tensor("W1", (H0, H1), bf16, kind="ExternalInput")
    W2 = nc.dram_tensor("W2", (H1, OUT), bf16, kind="ExternalInput")
    # columns: [w_fc, w_fe, w_ef, w_cf, w_ef, w_fe]
    wHE = nc.dram_tensor("wHE", (D, 6), bf16, kind="ExternalInput")
    bfu = nc.dram_tensor("bfu", (D, 2), f32, kind="ExternalInput")
    b0r = nc.dram_tensor("b0r", (D, 4), f32, kind="ExternalInput")
    b1r = nc.dram_tensor("b1r", (D, 2), f32, kind="ExternalInput")
    b2r = nc.dram_tensor("b2r", (OUT, 1), f32, kind="ExternalInput")
    bce = nc.dram_tensor("bce", (D, 2), f32, kind="ExternalInput")
    prob = nc.dram_tensor("prob", (BC, OUT), f32, kind="ExternalOutput")

    with tile.TileContext(nc) as tc:
        with (
            tc.tile_pool(name="pers", bufs=1) as pers,
            tc.tile_pool(name="work", bufs=3) as work,
            tc.tile_pool(name="ps", bufs=4, space="PSUM") as psp,
        ):
            # ---- index load first: the gathers are the long serial pole ----
            ix_t = pers.tile([128, NT], i32, tag="ix")
            nc.sync.dma_start(out=ix_t[:], in_=idx2[:])

            # ---- persistent weights / constants ----
            wf_t = pers.tile([128, F // 128, D], bf16, tag="wf")
            nc.sync.dma_start(out=wf_t[:], in_=Wf.rearrange("(a p) d -> p a d", p=128))
            wu_t = pers.tile([128, D], bf16, tag="wu")
            nc.sync.dma_start(out=wu_t[:], in_=Wu[:])
            w0_t = pers.tile([128, 2, H0], bf16, tag="w0")
            nc.sync.dma_start(out=w0_t[:], in_=W0.rearrange("(a p) h -> p a h", p=128))
            w1_t = pers.tile([128, H0 // 128, H1], bf16, tag="w1")
            nc.sync.dma_start(out=w1_t[:], in_=W1.rearrange("(a p) h -> p a h", p=128))
            w2_t = pers.tile([128, H1 // 128, OUT], bf16, tag="w2")
            nc.sync.dma_start(out=w2_t[:], in_=W2.rearrange("(a p) o -> p a o", p=128))
            whe_t = pers.tile([128, 6], bf16, tag="whe")
            nc.sync.dma_start(out=whe_t[:], in_=wHE[:])
            bfu_t = pers.tile([128, 2], f32, tag="bfu")
            nc.sync.dma_start(out=bfu_t[:], in_=bfu[:])
            b0_t = pers.tile([128, 4], f32, tag="b0")
            nc.sync.dma_start(out=b0_t[:], in_=b0r[:])
            b1_t = pers.tile([128, 2], f32, tag="b1")
            nc.sync.dma_start(out=b1_t[:], in_=b1r[:])
            b2_t = pers.tile([OUT, 1], f32, tag="b2")
            nc.sync.dma_start(out=b2_t[:], in_=b2r[:])
            bce_t = pers.tile([128, 2], f32, tag="bce")
            nc.sync.dma_start(out=bce_t[:], in_=bce[:])

            ident = pers.tile([128, 128], f32, tag="ident")
            make_identity(nc, ident[:])
            identb = pers.tile([128, 128], bf16, tag="identb")
            nc.vector.tensor_copy(out=identb[:], in_=ident[:])

            # ---- per-tile gathers; head/ent interleaved -> 1KB per descriptor ----
            he_all = pers.tile([128, NT * 2 * D], bf16, tag="he")
            for t in range(NT):
                nc.gpsimd.indirect_dma_start(
                    out=he_all[:, t * 2 * D:(t + 1) * 2 * D],
                    out_offset=None, in_=tabs[:],
                    in_offset=bass.IndirectOffsetOnAxis(ap=ix_t[:, t:t + 1], axis=0))

            xT = pers.tile([128, F // 128, BC], bf16, tag="xT")
            nc.sync.dma_start(out=xT[:], in_=featT.rearrange("(a p) b -> p a b", p=128))

            # ---- column sums of wHE, broadcast to all partitions ----
            ones_c = pers.tile([128, 1], bf16, tag="ones_c")
            nc.vector.memset(ones_c[:], 1.0)
            ones_r = pers.tile([1, 128], bf16, tag="ones_r")
            nc.vector.memset(ones_r[:], 1.0)
            sums_ps = psp.tile([1, 6], f32, tag="smps", bufs=2)
            nc.tensor.matmul(out=sums_ps[:], lhsT=ones_c[:], rhs=whe_t[:],
                             start=True, stop=True)
            sums_sb = pers.tile([1, 6], bf16, tag="sums")
            nc.scalar.activation(out=sums_sb[:], in_=sums_ps[:], func=AF.Copy)
            sb_ps = psp.tile([128, 6], f32, tag="smps", bufs=2)
            nc.tensor.matmul(out=sb_ps[:], lhsT=ones_r[:], rhs=sums_sb[:],
                             start=True, stop=True)
            sb_t = pers.tile([128, 6], bf16, tag="sb")
            nc.scalar.activation(out=sb_t[:], in_=sb_ps[:], func=AF.Copy)
            # cef = b_e * sum(w_ef); cfe = b_c * sum(w_fe)
            cef = pers.tile([128, 1], f32, tag="cef")
            nc.vector.tensor_tensor(out=cef[:], in0=sb_t[:, 2:3], in1=bce_t[:, 1:2],
                                    op=OP.mult)
            cfe = pers.tile([128, 1], f32, tag="cfe")
            nc.vector.tensor_tensor(out=cfe[:], in0=sb_t[:, 1:2], in1=bce_t[:, 0:1],
                                    op=OP.mult)

            # ---- transpose h0/e0 into [d, b]; batched PSUM -> one copy per 512 ----
            def h0s(t):
                return he_all[:, t * 2 * D:t * 2 * D + D]

            def e0s(t):
                return he_all[:, t * 2 * D + D:(t + 1) * 2 * D]

            hT_all = pers.tile([128, BC], bf16, tag="hT")
            eT_all = pers.tile([128, BC], bf16, tag="eT")
            for g in range(NG):
                gs = slice(g * 512, (g + 1) * 512)
                for sel, dst in ((h0s, hT_all), (e0s, eT_all)):
                    tp = psp.tile([128, 512], bf16, tag="trps", bufs=2)
                    for j in range(GT):
                        t = g * GT + j
                        nc.tensor.transpose(
                            out=tp[:, j * 128:(j + 1) * 128],
                            in_=sel(t), identity=identb[:])
                    nc.scalar.activation(out=dst[:, gs], in_=tp[:], func=AF.Copy)

            # ---- per-row dot products (fp32r needs even moving dim -> N=4) ----
            # cols 0..3 = h0 . (w_fc, w_fe, w_ef, w_cf)
            # cols 4..7 = e0 . (w_ef, w_cf, w_ef, w_fe)
            dots = pers.tile([128, NT * 8], bf16, tag="dots")
            for t in range(NT):
                bs = slice(t * 128, (t + 1) * 128)
                d_ps = psp.tile([128, 8], f32, tag="smps", bufs=2)
                nc.tensor.matmul(out=d_ps[:, 0:4], lhsT=hT_all[:, bs],
                                 rhs=whe_t[:, 0:4], start=True, stop=True)
                nc.tensor.matmul(out=d_ps[:, 4:8], lhsT=eT_all[:, bs],
                                 rhs=whe_t[:, 2:6], start=True, stop=True)
                nc.vector.tensor_copy(out=dots[:, t * 8:(t + 1) * 8], in_=d_ps[:, 0:8])

            # ---- coefficients A, B, C  (views strided across tiles) ----
            dv = dots.rearrange("p (t c) -> p t c", c=8)
            c0, c1, c2 = dv[:, :, 0], dv[:, :, 1], dv[:, :, 2]  # H_fc, H_fe, H_ef
            c3, c4, c5 = dv[:, :, 5], dv[:, :, 4], dv[:, :, 7]  # E_cf, E_ef, E_fe

            def tt(out, a, b, op):
                nc.vector.tensor_tensor(out=out, in0=a, in1=b, op=op)

            t1 = pers.tile([128, NT], bf16, tag="t1")
            t2 = pers.tile([128, NT], bf16, tag="t2")
            a2 = pers.tile([128, NT], bf16, tag="a2")
            d2 = pers.tile([128, NT], bf16, tag="d2")
            A = pers.tile([128, NT], f32, tag="A")
            Bc = pers.tile([128, NT], f32, tag="B")
            Cc = pers.tile([128, NT], f32, tag="C")
            # a2 = c4*c2 + c1*c4 + cef
            tt(t1[:], c4, c2, OP.mult)
            tt(t2[:], c1, c4, OP.mult)
            tt(a2[:], t1[:], t2[:], OP.add)
            nc.vector.tensor_scalar(out=a2[:], in0=a2[:], scalar1=cef[:, 0:1],
                                    scalar2=None, op0=OP.add)
            # d2 = c3*c1 + c0*c5 + cfe
            tt(t1[:], c3, c1, OP.mult)
            tt(t2[:], c0, c5, OP.mult)
            tt(d2[:], t1[:], t2[:], OP.add)
            nc.vector.tensor_scalar(out=d2[:], in0=d2[:], scalar1=cfe[:, 0:1],
                                    scalar2=None, op0=OP.add)
            # A = a2*c3 + d2*c4 ; B = a2*c0 + d2*c1
            tt(t1[:], a2[:], c3, OP.mult)
            tt(t2[:], d2[:], c4, OP.mult)
            tt(A[:], t1[:], t2[:], OP.add)
            tt(t1[:], a2[:], c0, OP.mult)
            tt(t2[:], d2[:], c1, OP.mult)
            tt(Bc[:], t1[:], t2[:], OP.add)
            # C = a2*b_c + (d2*b_e + b_e)
            nc.vector.tensor_scalar(out=t1[:], in0=a2[:], scalar1=bce_t[:, 0:1],
                                    scalar2=None, op0=OP.mult)
            nc.vector.tensor_scalar(out=t2[:], in0=d2[:], scalar1=bce_t[:, 1:2],
                                    scalar2=bce_t[:, 1:2], op0=OP.mult, op1=OP.add)
            tt(Cc[:], t1[:], t2[:], OP.add)

            # ---- e2 = A*h0 + B*e0 + C, transposed into [d, b] ----
            e2T = pers.tile([128, BC], bf16, tag="e2T")
            for g in range(NG):
                tp = psp.tile([128, 512], bf16, tag="trps", bufs=2)
                for j in range(GT):
                    t = g * GT + j
                    m1 = work.tile([128, 128], bf16, tag="m1")
                    nc.vector.tensor_scalar(out=m1[:], in0=h0s(t),
                                            scalar1=A[:, t:t + 1],
                                            scalar2=Cc[:, t:t + 1],
                                            op0=OP.mult, op1=OP.add)
                    e2n = work.tile([128, 128], bf16, tag="e2n")
                    nc.vector.tensor_scalar(out=e2n[:], in0=e0s(t),
                                            scalar1=Bc[:, t:t + 1], scalar2=None,
                                            op0=OP.mult)
                    tt(e2n[:], m1[:], e2n[:], OP.add)
                    nc.tensor.transpose(out=tp[:, j * 128:(j + 1) * 128],
                                        in_=e2n[:], identity=identb[:])
                nc.scalar.activation(out=e2T[:, g * 512:(g + 1) * 512], in_=tp[:],
                                     func=AF.Copy)

            # ---- main MLP, layer-major across groups to keep the PE fed ----
            GS = [slice(g * 512, (g + 1) * 512) for g in range(NG)]

            def relu(dst, src, bias_ap, on_vector):
                if on_vector:
                    nc.vector.tensor_scalar(out=dst, in0=src, scalar1=bias_ap,
                                            scalar2=0.0, op0=OP.add, op1=OP.max)
                else:
                    nc.scalar.activation(out=dst, in_=src, func=AF.Relu, bias=bias_ap)

            cf0 = pers.tile([128, BC], bf16, tag="cf0")
            cf1 = pers.tile([128, BC], bf16, tag="cf1")
            cf2 = pers.tile([128, BC], bf16, tag="cf2")
            x1a = pers.tile([128, NG, 4 * 512], bf16, tag="x1a")
            x2a = pers.tile([128, NG, 2 * 512], bf16, tag="x2a")
            x3a = pers.tile([OUT, NG, 512], f32, tag="x3a")
            for g in range(NG):
                cf_ps = psp.tile([128, 512], f32, tag="mmps", bufs=4)
                nc.tensor.matmul(out=cf_ps[:], lhsT=wf_t[:, 0, :], rhs=xT[:, 0, GS[g]],
                                 start=True, stop=False)
                nc.tensor.matmul(out=cf_ps[:], lhsT=wf_t[:, 1, :], rhs=xT[:, 1, GS[g]],
                                 start=False, stop=True)
                relu(cf0[:, GS[g]], cf_ps[:], bfu_t[:, 0:1], on_vector=(g % 2 == 1))
            for src, dst in ((cf0, cf1), (cf1, cf2)):
                for g in range(NG):
                    cu_ps = psp.tile([128, 512], f32, tag="mmps", bufs=4)
                    nc.tensor.matmul(out=cu_ps[:], lhsT=wu_t[:], rhs=src[:, GS[g]],
                                     start=True, stop=True)
                    relu(dst[:, GS[g]], cu_ps[:], bfu_t[:, 1:2],
                         on_vector=(g % 2 == 1))
            for g in range(NG):
                for m in range(4):
                    x1_ps = psp.tile([128, 512], f32, tag="mmps", bufs=4)
                    ms = slice(m * 128, (m + 1) * 128)
                    nc.tensor.matmul(out=x1_ps[:], lhsT=w0_t[:, 0, ms],
                                     rhs=cf2[:, GS[g]], start=True, stop=False)
                    nc.tensor.matmul(out=x1_ps[:], lhsT=w0_t[:, 1, ms],
                                     rhs=e2T[:, GS[g]], start=False, stop=True)
                    relu(x1a[:, g, m * 512:(m + 1) * 512], x1_ps[:], b0_t[:, m:m + 1],
                         on_vector=(m % 2 == 1))
            for g in range(NG):
                for m in range(2):
                    x2_ps = psp.tile([128, 512], f32, tag="mmps", bufs=4)
                    ms = slice(m * 128, (m + 1) * 128)
                    for k in range(4):
                        nc.tensor.matmul(out=x2_ps[:], lhsT=w1_t[:, k, ms],
                                         rhs=x1a[:, g, k * 512:(k + 1) * 512],
                                         start=(k == 0), stop=(k == 3))
                    relu(x2a[:, g, m * 512:(m + 1) * 512], x2_ps[:], b1_t[:, m:m + 1],
                         on_vector=(m % 2 == 1))
            # x3 + softmax per group (transpose back to [b, 8] in 128-row blocks)
            for g in range(NG):
                x3_ps = psp.tile([OUT, 512], f32, tag="smps", bufs=2)
                for k in range(2):
                    nc.tensor.matmul(out=x3_ps[:], lhsT=w2_t[:, k, :],
                                     rhs=x2a[:, g, k * 512:(k + 1) * 512],
                                     start=(k == 0), stop=(k == 1))
                nc.scalar.activation(out=x3a[:, g, :], in_=x3_ps[:], func=AF.Relu,
                                     bias=b2_t[:, 0:1])
                ex = work.tile([128, GT * OUT], f32, tag="ex")
                for j in range(GT):
                    sm_ps = psp.tile([128, OUT], f32, tag="smps", bufs=2)
                    nc.tensor.transpose(out=sm_ps[:],
                                        in_=x3a[:, g, j * 128:(j + 1) * 128],
                                        identity=ident[:OUT, :OUT])
                    nc.scalar.activation(out=ex[:, j * OUT:(j + 1) * OUT],
                                         in_=sm_ps[:], func=AF.Exp)
                sm_sum = work.tile([128, GT], f32, tag="sm_sum")
                nc.vector.reduce_sum(out=sm_sum[:],
                                     in_=ex.rearrange("p (j o) -> p j o", o=OUT),
                                     axis=AX.X)
                sm_rec = work.tile([128, GT], f32, tag="sm_rec")
                nc.vector.reciprocal(out=sm_rec[:], in_=sm_sum[:])
                pr = work.tile([128, GT * OUT], f32, tag="pr")
                nc.vector.tensor_tensor(
                    out=pr.rearrange("p (j o) -> p j o", o=OUT),
                    in0=ex.rearrange("p (j o) -> p j o", o=OUT),
                    in1=sm_rec.rearrange("p (j o) -> p j o", o=1).to_broadcast(
                        [128, GT, OUT]),
                    op=OP.mult)
                nc.sync.dma_start(
                    out=prob[g * 512:(g + 1) * 512, :].rearrange(
                        "(j p) o -> p j o", p=128),
                    in_=pr.rearrange("p (j o) -> p j o", o=OUT))

    nc.finalize()
    return nc


def _get_nc():
    if "nc" not in _CACHE:
        _CACHE["nc"] = _build()
    return _CACHE["nc"]


def kernel(features, ent_idx, target, Wf, bf, Wu, bu, w_cf, w_fc, w_ef, w_fe,
           b_c, b_e, head_tab, ent_tab, W0, b0, W1, b1, W2, b2):
    from concourse.bass_utils import run_bass_kernel_spmd

    import ml_dtypes
    bf16 = ml_dtypes.bfloat16
    features = np.asarray(features, dtype=np.float32)
    ent_idx = np.asarray(ent_idx)
    target = np.asarray(target)
    f32 = np.float32
    head_tab = np.asarray(head_tab, dtype=f32)
    ent_tab = np.asarray(ent_tab, dtype=f32)
    w_cf, w_fc = np.asarray(w_cf, f32), np.asarray(w_fc, f32)
    w_ef, w_fe = np.asarray(w_ef, f32), np.asarray(w_fe, f32)

    featT = np.ascontiguousarray(features.T.astype(bf16))                    # [F, B]
    idx_all = ent_idx.astype(np.int32)
    wHE = np.ascontiguousarray(
        np.stack([w_fc, w_fe, w_ef, w_cf, w_ef, w_fe], axis=1))  # [D, 6]
    bfu = np.ascontiguousarray(
        np.stack([np.asarray(bf, f32), np.asarray(bu, f32)], axis=1))
    b0r = np.ascontiguousarray(np.asarray(b0, f32).reshape(4, D).T)
    b1r = np.ascontiguousarray(np.asarray(b1, f32).reshape(2, D).T)
    b2r = np.ascontiguousarray(np.asarray(b2, f32).reshape(OUT, 1))
    bce = np.ascontiguousarray(np.broadcast_to(
        np.array([np.asarray(b_c, f32).reshape(()),
                  np.asarray(b_e, f32).reshape(())], dtype=f32), (D, 2)))
    tabs = np.ascontiguousarray(np.concatenate([head_tab, ent_tab], axis=1))
    shared = dict(
        tabs=tabs,
        Wf=np.asarray(Wf, bf16), Wu=np.asarray(Wu, bf16),
        W0=np.asarray(W0, bf16), W1=np.asarray(W1, bf16), W2=np.asarray(W2, bf16),
        wHE=wHE.astype(bf16), bfu=bfu, b0r=b0r, b1r=b1r, b2r=b2r, bce=bce,
    )
    in_maps = []
    for c in range(NCORES):
        cs = slice(c * BC, (c + 1) * BC)
        in_maps.append(dict(
            featT=np.ascontiguousarray(featT[:, cs]),
            idx2=np.ascontiguousarray(idx_all[cs].reshape(NT, 128).T),
            **shared))

    nc = _get_nc()
    res = run_bass_kernel_spmd(nc, in_maps, core_ids=list(range(NCORES)))
    prob = np.concatenate([r["prob"] for r in res.results], axis=0)
    return prob, target
